# revision 23
# baseline (speedup 1.0000x reference)
"""Trainium2 Bass kernel for nn_NeuralUniLasso (dense_mlp).

Per-feature tiny MLPs: h1 = tanh(x*W1+b1) [B,F,12]; h2 = tanh(h1@W2+b2)
[B,F,8]; Z = h2@W3+b3 [B,F]; weights = softplus(theta); out = Z@weights+bias.

Strategy (per core, data-parallel over batch, B_s = 4096):
  - Layout: features on partitions, batch on free dim (xT [F, B_s], fp16).
  - mm1 (PE): 0/1 replication matrices expand xT [128f, NB] into the
    (f,i)-interleaved layout [128 rows of 12f+i, NB] (48 groups cover
    512*12 = 6144 rows). ACT then computes tanh(W1*x + b1) in ONE pass per
    group using its per-partition scale/bias FMA -> h1 (fp16).
  - mm2 (PE): block-diag W2 matrices [128 (f,i), 128 (f,j)] contract i;
    2 accumulating matmuls per M-tile (K-groups straddle M boundaries).
    ACT: tanh(psum + b2vec) -> h2 (fp16).
  - mm3 (PE): W3 block matrices [128 (f,j), 32 f] contract j, col-tiled
    into a [128f, NB] psum via tile_position. ACT: Identity + b3vec -> Z.
  - mm4 (PE): out = Z @ softplus(theta), 3-pass hi/lo fp16 split for
    fp32-grade accuracy; +bias via ACT.
All engine work: ACT ~330us (bottleneck), PE ~270us, DVE/DMA small.
"""

import os
import numpy as np

B, F, H1, H2 = 32768, 512, 12, 8
NCORES = 8
BS = B // NCORES          # 4096 batch per core
NB = 1024                 # batch chunk (free dim)
NCH = BS // NB            # 2 chunks per core
NG = (F * H1) // 128      # 48 groups of 128 (f,i)-rows
NM = (F * H2) // 128      # 32 M-tiles of 128 (f,j)-rows
NT = F // 128             # 4 feature tiles

_cache = {}


def _g_pair(m):
    # K-groups feeding M-tile m: m=2u -> (3u, 3u+1); m=2u+1 -> (3u+1, 3u+2)
    u, r = divmod(m, 2)
    return (3 * u, 3 * u + 1) if r == 0 else (3 * u + 1, 3 * u + 2)


def _build_nc():
    import concourse.bass as bass
    import concourse.bacc as bacc
    import concourse.tile as tile
    import concourse.mybir as mybir

    f32, f16 = mybir.dt.float32, mybir.dt.float16
    AF = mybir.ActivationFunctionType

    nc = bacc.Bacc()
    xT_d = nc.dram_tensor("xT16", [F, BS], f16, kind="ExternalInput")
    A_d = nc.dram_tensor("Arep", [128, NG, 128], f16, kind="ExternalInput")
    W2_d = nc.dram_tensor("W2blk", [128, 2 * NM, 128], f16, kind="ExternalInput")
    W3_d = nc.dram_tensor("W3blk", [128, NM, 32], f16, kind="ExternalInput")
    w1v_d = nc.dram_tensor("w1v", [128, NG], f32, kind="ExternalInput")
    b1v_d = nc.dram_tensor("b1v", [128, NG], f32, kind="ExternalInput")
    b2v_d = nc.dram_tensor("b2v", [128, NM], f32, kind="ExternalInput")
    b3v_d = nc.dram_tensor("b3v", [128, NT], f32, kind="ExternalInput")
    thv_d = nc.dram_tensor("thv", [128, NT], f32, kind="ExternalInput")
    bias_d = nc.dram_tensor("biasv", [1, 1], f32, kind="ExternalInput")

    ZT_d = nc.dram_tensor("ZT", [F, BS], f32, kind="ExternalOutput")
    out_d = nc.dram_tensor("outv", [BS], f32, kind="ExternalOutput")
    wts_d = nc.dram_tensor("wts", [128, NT], f32, kind="ExternalOutput")

    with tile.TileContext(nc) as tc:
        import contextlib

        ctx = contextlib.ExitStack()
        with ctx:
            consts = ctx.enter_context(tc.tile_pool(name="consts", bufs=1))
            xpool = ctx.enter_context(tc.tile_pool(name="xpool", bufs=5))
            h1pool = ctx.enter_context(tc.tile_pool(name="h1pool", bufs=6))
            h2pool = ctx.enter_context(tc.tile_pool(name="h2pool", bufs=9))
            zpool = ctx.enter_context(tc.tile_pool(name="zpool", bufs=3))
            zhip = ctx.enter_context(tc.tile_pool(name="zhip", bufs=4))
            zlop = ctx.enter_context(tc.tile_pool(name="zlop", bufs=4))
            outp = ctx.enter_context(tc.tile_pool(name="outp", bufs=2))
            psA = ctx.enter_context(tc.tile_pool(name="psA", bufs=2, space="PSUM"))
            psB = ctx.enter_context(tc.tile_pool(name="psB", bufs=1, space="PSUM"))
            psC = ctx.enter_context(tc.tile_pool(name="psC", bufs=1, space="PSUM"))
            psD = ctx.enter_context(tc.tile_pool(name="psD", bufs=1, space="PSUM"))

            # ---- constants into SBUF ----
            # small scale/bias vectors first (first h1 op depends on them)
            w1v = consts.tile([128, NG], f32)
            nc.gpsimd.dma_start(out=w1v, in_=w1v_d[:, :])
            b1v = consts.tile([128, NG], f32)
            nc.gpsimd.dma_start(out=b1v, in_=b1v_d[:, :])
            b2v = consts.tile([128, NM], f32)
            nc.gpsimd.dma_start(out=b2v, in_=b2v_d[:, :])
            b3v = consts.tile([128, NT], f32)
            nc.gpsimd.dma_start(out=b3v, in_=b3v_d[:, :])
            thv = consts.tile([128, NT], f32)
            nc.gpsimd.dma_start(out=thv, in_=thv_d[:, :])
            biasv = consts.tile([1, 1], f32)
            nc.gpsimd.dma_start(out=biasv, in_=bias_d[:, :])
            # interleave chunk-0 x tiles with A slices: mm1 group g needs
            # xt[g//12] and A quarter g//12, in that order, ASAP
            A_sb = consts.tile([128, NG, 128], f16)
            xt0 = []
            for t in range(NT):
                xt_t = xpool.tile([128, NB], f16, tag="xt")
                nc.gpsimd.dma_start(
                    out=xt_t, in_=xT_d[128 * t : 128 * (t + 1), 0:NB]
                )
                xt0.append(xt_t)
                sl = slice(t * NG // 4, (t + 1) * NG // 4)
                nc.gpsimd.dma_start(out=A_sb[:, sl, :], in_=A_d[:, sl, :])
            W2_sb = consts.tile([128, 2 * NM, 128], f16)
            for q in range(4):
                sl = slice(q * NM // 2, (q + 1) * NM // 2)
                nc.gpsimd.dma_start(out=W2_sb[:, sl, :], in_=W2_d[:, sl, :])
            W3_sb = consts.tile([128, NM, 32], f16)
            nc.gpsimd.dma_start(out=W3_sb, in_=W3_d[:, :, :])

            # softplus(theta) = ln(1+exp(theta)) once, before the Tanh
            # stream (Softplus has no ACT table set in this build)
            esb = consts.tile([128, NT], f32)
            nc.scalar.activation(out=esb, in_=thv, func=AF.Exp)
            e1sb = consts.tile([128, NT], f32)
            nc.vector.tensor_scalar_add(out=e1sb, in0=esb, scalar1=1.0)
            wsb = consts.tile([128, NT], f32)
            nc.scalar.activation(out=wsb, in_=e1sb, func=AF.Ln)
            nc.gpsimd.dma_start(out=wts_d[:, :], in_=wsb[:, :])
            whi = consts.tile([128, NT], f16)
            nc.vector.tensor_copy(out=whi, in_=wsb)
            wlo = consts.tile([128, NT], f16)
            nc.vector.tensor_tensor(
                out=wlo, in0=wsb, in1=whi, op=mybir.AluOpType.subtract
            )

            for c in range(NCH):
                if c == 0:
                    xt = xt0
                else:
                    xt = []
                    for t in range(NT):
                        xt_t = xpool.tile([128, NB], f16, tag="xt")
                        nc.gpsimd.dma_start(
                            out=xt_t,
                            in_=xT_d[128 * t : 128 * (t + 1), c * NB : (c + 1) * NB],
                        )
                        xt.append(xt_t)

                h1 = {}
                h2 = {}
                zhi, zlo = [], []
                outsb = outp.tile([1, NB], f32, tag="out")
                for u in range(16):
                    for g in (3 * u, 3 * u + 1, 3 * u + 2):
                        h1_g = h1pool.tile([128, NB], f16, tag="h1")
                        # psumA double-buffered at half width: mm1 for the
                        # next half runs while ACT drains the previous one
                        for half in range(NB // 1024):
                            pa = psA.tile([128, 1024], f32)
                            for s in range(2):
                                off = 1024 * half + 512 * s
                                nc.tensor.matmul(
                                    pa[:, 512 * s : 512 * (s + 1)],
                                    A_sb[:, g, :],
                                    xt[g // 12][:, off : off + 512],
                                    start=True,
                                    stop=True,
                                )
                            nc.scalar.activation(
                                out=h1_g[:, 1024 * half : 1024 * (half + 1)],
                                in_=pa,
                                func=AF.Tanh,
                                bias=b1v[:, g : g + 1],
                                scale=w1v[:, g : g + 1],
                            )
                        h1[g] = h1_g
                    for m in (2 * u, 2 * u + 1):
                        g0, g1 = _g_pair(m)
                        h2_m = h2pool.tile([128, NB], f16, tag="h2")
                        for half in range(NB // 1024):
                            pb = psB.tile([128, 1024], f32)
                            for s2 in range(2):
                                off = 1024 * half + 512 * s2
                                sl = slice(512 * s2, 512 * (s2 + 1))
                                nc.tensor.matmul(
                                    pb[:, sl],
                                    W2_sb[:, 2 * m, :],
                                    h1[g0][:, off : off + 512],
                                    start=True,
                                    stop=False,
                                )
                                nc.tensor.matmul(
                                    pb[:, sl],
                                    W2_sb[:, 2 * m + 1, :],
                                    h1[g1][:, off : off + 512],
                                    start=False,
                                    stop=True,
                                )
                            nc.scalar.activation(
                                out=h2_m[:, 1024 * half : 1024 * (half + 1)],
                                in_=pb,
                                func=AF.Tanh,
                                bias=b2v[:, m : m + 1],
                            )
                        h2[m] = h2_m
                    if u % 4 == 3:
                        t = u // 4
                        zt = zpool.tile([128, NB], f32, tag="z")
                        for s in range(NB // 512):
                            pc = psC.tile([128, 512], f32)
                            for mi in range(8):
                                m = 8 * t + mi
                                c0 = mi // 2
                                nc.tensor.matmul(
                                    pc[32 * c0 : 32 * (c0 + 1), :],
                                    W3_sb[:, m, :],
                                    h2[m][:, 512 * s : 512 * (s + 1)],
                                    start=(mi % 2 == 0),
                                    stop=(mi % 2 == 1),
                                    tile_position=(0, 32 * c0),
                                )
                            # drain Z psum on DVE (ACT is the bottleneck)
                            nc.vector.tensor_scalar_add(
                                out=zt[:, 512 * s : 512 * (s + 1)],
                                in0=pc,
                                scalar1=b3v[:, t : t + 1],
                            )
                        nc.gpsimd.dma_start(
                            out=ZT_d[128 * t : 128 * (t + 1), c * NB : (c + 1) * NB],
                            in_=zt,
                        )
                        zh = zhip.tile([128, NB], f16, tag="zhi")
                        nc.vector.tensor_copy(out=zh, in_=zt)
                        zl = zlop.tile([128, NB], f16, tag="zlo")
                        nc.vector.tensor_tensor(
                            out=zl, in0=zt, in1=zh, op=mybir.AluOpType.subtract
                        )
                        # mm4 partial dot per t, spread through the chunk:
                        # psD drains into outsb via DVE accumulation
                        for s in range(NB // 512):
                            pd = psD.tile([1, 512], f32)
                            for k, (lh, rh) in enumerate(
                                ((whi, zh), (whi, zl), (wlo, zh))
                            ):
                                nc.tensor.matmul(
                                    pd[:, :],
                                    lh[:, t : t + 1],
                                    rh[:, 512 * s : 512 * (s + 1)],
                                    start=(k == 0),
                                    stop=(k == 2),
                                )
                            osl = outsb[:, 512 * s : 512 * (s + 1)]
                            if t == 0:
                                nc.vector.tensor_scalar_add(
                                    out=osl, in0=pd, scalar1=biasv
                                )
                            else:
                                nc.vector.tensor_tensor(
                                    out=osl,
                                    in0=osl,
                                    in1=pd,
                                    op=mybir.AluOpType.add,
                                )

                nc.gpsimd.dma_start(
                    out=out_d[c * NB : (c + 1) * NB].rearrange(
                        "(one n) -> one n", one=1
                    ),
                    in_=outsb,
                )
    nc.finalize()
    return nc


def _build_weights(W1, b1, W2, b2, W3, b3, theta, bias):
    f16 = np.float16
    # A: 0/1 replication matrices. A[k, g, mcol] = 1 iff feature of row
    # (128g + mcol) is local feature k of f-tile g//12.
    gg, mm = np.meshgrid(np.arange(NG), np.arange(128), indexing="ij")
    r = 128 * gg + mm
    f = r // H1
    k = f - 128 * (gg // 12)
    A = np.zeros((128, NG, 128), dtype=f16)
    A[k.ravel(), gg.ravel(), mm.ravel()] = 1.0

    # W2blk[k, 2m+w, q]
    W2blk = np.zeros((128, 2 * NM, 128), dtype=f16)
    q = np.arange(128)
    f_s = q[None, :] * 0  # placeholder
    for m in range(NM):
        s = 128 * m + q
        f_col, j_col = s // H2, s % H2
        for w, g in enumerate(_g_pair(m)):
            rr = 128 * g + np.arange(128)
            f_row, i_row = rr // H1, rr % H1
            mask = f_row[:, None] == f_col[None, :]
            vals = W2[f_row[:, None], i_row[:, None], j_col[None, :]].astype(np.float32)
            W2blk[:, 2 * m + w, :] = np.where(mask, vals, 0.0).astype(f16)

    # W3blk[q, m, c]
    W3blk = np.zeros((128, NM, 32), dtype=f16)
    cc = np.arange(32)
    for m in range(NM):
        s = 128 * m + q
        f_col, j_col = s // H2, s % H2
        fo = 128 * (m // 8) + 32 * ((m % 8) // 2) + cc
        mask = f_col[:, None] == fo[None, :]
        vals = W3[f_col, j_col].astype(np.float32)[:, None]
        W3blk[:, m, :] = np.where(mask, vals, 0.0).astype(f16)

    w1v = np.ascontiguousarray(
        W1.astype(np.float32).ravel().reshape(NG, 128).T
    )
    b1v = np.ascontiguousarray(b1.astype(np.float32).ravel().reshape(NG, 128).T)
    b2v = np.ascontiguousarray(b2.astype(np.float32).ravel().reshape(NM, 128).T)
    b3v = np.ascontiguousarray(b3.astype(np.float32).reshape(NT, 128).T)
    thv = np.ascontiguousarray(theta.astype(np.float32).reshape(NT, 128).T)
    biasv = np.asarray(bias, dtype=np.float32).reshape(1, 1)
    return dict(
        Arep=A, W2blk=W2blk, W3blk=W3blk, w1v=w1v, b1v=b1v, b2v=b2v,
        b3v=b3v, thv=thv, biasv=biasv,
    )


def kernel(x, W1, b1, W2, b2, W3, b3, theta, bias):
    from concourse.bass_utils import run_bass_kernel_spmd

    x = np.asarray(x, dtype=np.float32)
    W1 = np.asarray(W1); b1 = np.asarray(b1); W2 = np.asarray(W2)
    b2 = np.asarray(b2); W3 = np.asarray(W3); b3 = np.asarray(b3)
    theta = np.asarray(theta); bias = np.asarray(bias)

    if "nc" not in _cache:
        _cache["nc"] = _build_nc()
    nc = _cache["nc"]

    wd = _build_weights(W1, b1, W2, b2, W3, b3, theta, bias)
    xT16 = np.ascontiguousarray(x.T.astype(np.float16))  # [F, B]

    in_maps = []
    for cidx in range(NCORES):
        m = dict(wd)
        m["xT16"] = np.ascontiguousarray(xT16[:, cidx * BS : (cidx + 1) * BS])
        in_maps.append(m)

    trace = bool(os.environ.get("BASS_KERNEL_TRACE"))
    res = run_bass_kernel_spmd(
        nc, in_maps, core_ids=list(range(NCORES)), trace=trace
    )
    _cache["last_exec_ns"] = res.exec_time_ns
    _cache["last_res"] = res

    Z = np.empty((B, F), dtype=np.float32)
    out = np.empty((B,), dtype=np.float32)
    for cidx in range(NCORES):
        r = res.results[cidx]
        Z[cidx * BS : (cidx + 1) * BS, :] = r["ZT"].T
        out[cidx * BS : (cidx + 1) * BS] = r["outv"]
    weights = np.ascontiguousarray(res.results[0]["wts"].T).reshape(F).astype(np.float32)
    return out, weights, Z


# revision 24
# speedup vs baseline: 1.0815x; 1.0815x over previous
"""Trainium2 Bass kernel for nn_NeuralUniLasso (dense_mlp).

Per-feature tiny MLPs: h1 = tanh(x*W1+b1) [B,F,12]; h2 = tanh(h1@W2+b2)
[B,F,8]; Z = h2@W3+b3 [B,F]; weights = softplus(theta); out = Z@weights+bias.

Strategy (per core, data-parallel over batch, B_s = 4096):
  - Layout: features on partitions, batch on free dim (xT [F, B_s], fp16).
  - mm1 (PE): 0/1 replication matrices expand xT [128f, NB] into the
    (f,i)-interleaved layout [128 rows of 12f+i, NB] (48 groups cover
    512*12 = 6144 rows). ACT then computes tanh(W1*x + b1) in ONE pass per
    group using its per-partition scale/bias FMA -> h1 (fp16).
  - mm2 (PE): block-diag W2 matrices [128 (f,i), 128 (f,j)] contract i;
    2 accumulating matmuls per M-tile (K-groups straddle M boundaries).
    ACT: tanh(psum + b2vec) -> h2 (fp16).
  - mm3 (PE): W3 block matrices [128 (f,j), 32 f] contract j, col-tiled
    into a [128f, NB] psum via tile_position. ACT: Identity + b3vec -> Z.
  - mm4 (PE): out = Z @ softplus(theta), 3-pass hi/lo fp16 split for
    fp32-grade accuracy; +bias via ACT.
All engine work: ACT ~330us (bottleneck), PE ~270us, DVE/DMA small.
"""

import os
import numpy as np

B, F, H1, H2 = 32768, 512, 12, 8
NCORES = 8
BS = B // NCORES          # 4096 batch per core
NB = 2048                 # batch chunk (free dim)
NCH = BS // NB            # 2 chunks per core
NG = (F * H1) // 128      # 48 groups of 128 (f,i)-rows
NM = (F * H2) // 128      # 32 M-tiles of 128 (f,j)-rows
NT = F // 128             # 4 feature tiles

_cache = {}


def _g_pair(m):
    # K-groups feeding M-tile m: m=2u -> (3u, 3u+1); m=2u+1 -> (3u+1, 3u+2)
    u, r = divmod(m, 2)
    return (3 * u, 3 * u + 1) if r == 0 else (3 * u + 1, 3 * u + 2)


def _build_nc():
    import concourse.bass as bass
    import concourse.bacc as bacc
    import concourse.tile as tile
    import concourse.mybir as mybir

    f32, f16 = mybir.dt.float32, mybir.dt.float16
    AF = mybir.ActivationFunctionType

    nc = bacc.Bacc()
    xT_d = nc.dram_tensor("xT16", [F, BS], f16, kind="ExternalInput")
    A_d = nc.dram_tensor("Arep", [128, NG, 128], f16, kind="ExternalInput")
    W2_d = nc.dram_tensor("W2blk", [128, 2 * NM, 128], f16, kind="ExternalInput")
    W3_d = nc.dram_tensor("W3blk", [128, NM, 32], f16, kind="ExternalInput")
    w1v_d = nc.dram_tensor("w1v", [128, NG], f32, kind="ExternalInput")
    b1v_d = nc.dram_tensor("b1v", [128, NG], f32, kind="ExternalInput")
    b2v_d = nc.dram_tensor("b2v", [128, NM], f32, kind="ExternalInput")
    b3v_d = nc.dram_tensor("b3v", [128, NT], f32, kind="ExternalInput")
    thv_d = nc.dram_tensor("thv", [128, NT], f32, kind="ExternalInput")
    bias_d = nc.dram_tensor("biasv", [1, 1], f32, kind="ExternalInput")

    ZT_d = nc.dram_tensor("ZT", [F, BS], f32, kind="ExternalOutput")
    out_d = nc.dram_tensor("outv", [BS], f32, kind="ExternalOutput")
    wts_d = nc.dram_tensor("wts", [128, NT], f32, kind="ExternalOutput")

    with tile.TileContext(nc) as tc:
        import contextlib

        ctx = contextlib.ExitStack()
        with ctx:
            consts = ctx.enter_context(tc.tile_pool(name="consts", bufs=1))
            xpool = ctx.enter_context(tc.tile_pool(name="xpool", bufs=5))
            h1pool = ctx.enter_context(tc.tile_pool(name="h1pool", bufs=8))
            h2pool = ctx.enter_context(tc.tile_pool(name="h2pool", bufs=11))
            zpool = ctx.enter_context(tc.tile_pool(name="zpool", bufs=3))
            zhip = ctx.enter_context(tc.tile_pool(name="zhip", bufs=4))
            zlop = ctx.enter_context(tc.tile_pool(name="zlop", bufs=4))
            outp = ctx.enter_context(tc.tile_pool(name="outp", bufs=2))
            psA = ctx.enter_context(tc.tile_pool(name="psA", bufs=2, space="PSUM"))
            psB = ctx.enter_context(tc.tile_pool(name="psB", bufs=1, space="PSUM"))
            psC = ctx.enter_context(tc.tile_pool(name="psC", bufs=1, space="PSUM"))
            psD = ctx.enter_context(tc.tile_pool(name="psD", bufs=1, space="PSUM"))

            # ---- constants into SBUF ----
            # small scale/bias vectors first (first h1 op depends on them)
            w1v = consts.tile([128, NG], f32)
            nc.gpsimd.dma_start(out=w1v, in_=w1v_d[:, :])
            b1v = consts.tile([128, NG], f32)
            nc.gpsimd.dma_start(out=b1v, in_=b1v_d[:, :])
            b2v = consts.tile([128, NM], f32)
            nc.gpsimd.dma_start(out=b2v, in_=b2v_d[:, :])
            b3v = consts.tile([128, NT], f32)
            nc.gpsimd.dma_start(out=b3v, in_=b3v_d[:, :])
            thv = consts.tile([128, NT], f32)
            nc.gpsimd.dma_start(out=thv, in_=thv_d[:, :])
            biasv = consts.tile([1, 1], f32)
            nc.gpsimd.dma_start(out=biasv, in_=bias_d[:, :])
            # interleave chunk-0 x tiles with A slices: mm1 group g needs
            # xt[g//12] and A quarter g//12, in that order, ASAP
            A_sb = consts.tile([128, NG, 128], f16)
            xt0 = []
            for t in range(NT):
                xt_t = xpool.tile([128, NB], f16, tag="xt")
                nc.gpsimd.dma_start(
                    out=xt_t, in_=xT_d[128 * t : 128 * (t + 1), 0:NB]
                )
                xt0.append(xt_t)
                sl = slice(t * NG // 4, (t + 1) * NG // 4)
                nc.gpsimd.dma_start(out=A_sb[:, sl, :], in_=A_d[:, sl, :])
            W2_sb = consts.tile([128, 2 * NM, 128], f16)
            for q in range(4):
                sl = slice(q * NM // 2, (q + 1) * NM // 2)
                nc.gpsimd.dma_start(out=W2_sb[:, sl, :], in_=W2_d[:, sl, :])
            W3_sb = consts.tile([128, NM, 32], f16)
            nc.gpsimd.dma_start(out=W3_sb, in_=W3_d[:, :, :])

            # softplus(theta) = ln(1+exp(theta)) once, before the Tanh
            # stream (Softplus has no ACT table set in this build)
            esb = consts.tile([128, NT], f32)
            nc.scalar.activation(out=esb, in_=thv, func=AF.Exp)
            e1sb = consts.tile([128, NT], f32)
            nc.vector.tensor_scalar_add(out=e1sb, in0=esb, scalar1=1.0)
            wsb = consts.tile([128, NT], f32)
            nc.scalar.activation(out=wsb, in_=e1sb, func=AF.Ln)
            nc.gpsimd.dma_start(out=wts_d[:, :], in_=wsb[:, :])
            whi = consts.tile([128, NT], f16)
            nc.vector.tensor_copy(out=whi, in_=wsb)
            wlo = consts.tile([128, NT], f16)
            nc.vector.tensor_tensor(
                out=wlo, in0=wsb, in1=whi, op=mybir.AluOpType.subtract
            )

            for c in range(NCH):
                if c == 0:
                    xt = xt0
                else:
                    xt = []
                    for t in range(NT):
                        xt_t = xpool.tile([128, NB], f16, tag="xt")
                        nc.gpsimd.dma_start(
                            out=xt_t,
                            in_=xT_d[128 * t : 128 * (t + 1), c * NB : (c + 1) * NB],
                        )
                        xt.append(xt_t)

                h1 = {}
                h2 = {}
                zhi, zlo = [], []
                outsb = outp.tile([1, NB], f32, tag="out")
                for u in range(16):
                    for g in (3 * u, 3 * u + 1, 3 * u + 2):
                        h1_g = h1pool.tile([128, NB], f16, tag="h1")
                        # psumA double-buffered at half width: mm1 for the
                        # next half runs while ACT drains the previous one
                        for half in range(NB // 1024):
                            pa = psA.tile([128, 1024], f32)
                            for s in range(2):
                                off = 1024 * half + 512 * s
                                nc.tensor.matmul(
                                    pa[:, 512 * s : 512 * (s + 1)],
                                    A_sb[:, g, :],
                                    xt[g // 12][:, off : off + 512],
                                    start=True,
                                    stop=True,
                                )
                            nc.scalar.activation(
                                out=h1_g[:, 1024 * half : 1024 * (half + 1)],
                                in_=pa,
                                func=AF.Tanh,
                                bias=b1v[:, g : g + 1],
                                scale=w1v[:, g : g + 1],
                            )
                        h1[g] = h1_g
                    for m in (2 * u, 2 * u + 1):
                        g0, g1 = _g_pair(m)
                        h2_m = h2pool.tile([128, NB], f16, tag="h2")
                        for half in range(NB // 1024):
                            pb = psB.tile([128, 1024], f32)
                            for s2 in range(2):
                                off = 1024 * half + 512 * s2
                                sl = slice(512 * s2, 512 * (s2 + 1))
                                nc.tensor.matmul(
                                    pb[:, sl],
                                    W2_sb[:, 2 * m, :],
                                    h1[g0][:, off : off + 512],
                                    start=True,
                                    stop=False,
                                )
                                nc.tensor.matmul(
                                    pb[:, sl],
                                    W2_sb[:, 2 * m + 1, :],
                                    h1[g1][:, off : off + 512],
                                    start=False,
                                    stop=True,
                                )
                            nc.scalar.activation(
                                out=h2_m[:, 1024 * half : 1024 * (half + 1)],
                                in_=pb,
                                func=AF.Tanh,
                                bias=b2v[:, m : m + 1],
                            )
                        h2[m] = h2_m
                    if u % 4 == 3:
                        t = u // 4
                        zt = zpool.tile([128, NB], f32, tag="z")
                        for s in range(NB // 512):
                            pc = psC.tile([128, 512], f32)
                            for mi in range(8):
                                m = 8 * t + mi
                                c0 = mi // 2
                                nc.tensor.matmul(
                                    pc[32 * c0 : 32 * (c0 + 1), :],
                                    W3_sb[:, m, :],
                                    h2[m][:, 512 * s : 512 * (s + 1)],
                                    start=(mi % 2 == 0),
                                    stop=(mi % 2 == 1),
                                    tile_position=(0, 32 * c0),
                                )
                            # drain Z psum on DVE (ACT is the bottleneck)
                            nc.vector.tensor_scalar_add(
                                out=zt[:, 512 * s : 512 * (s + 1)],
                                in0=pc,
                                scalar1=b3v[:, t : t + 1],
                            )
                        nc.gpsimd.dma_start(
                            out=ZT_d[128 * t : 128 * (t + 1), c * NB : (c + 1) * NB],
                            in_=zt,
                        )
                        zh = zhip.tile([128, NB], f16, tag="zhi")
                        nc.vector.tensor_copy(out=zh, in_=zt)
                        zl = zlop.tile([128, NB], f16, tag="zlo")
                        nc.vector.tensor_tensor(
                            out=zl, in0=zt, in1=zh, op=mybir.AluOpType.subtract
                        )
                        # mm4 partial dot per t, spread through the chunk:
                        # psD drains into outsb via DVE accumulation
                        for s in range(NB // 512):
                            pd = psD.tile([1, 512], f32)
                            for k, (lh, rh) in enumerate(
                                ((whi, zh), (whi, zl), (wlo, zh))
                            ):
                                nc.tensor.matmul(
                                    pd[:, :],
                                    lh[:, t : t + 1],
                                    rh[:, 512 * s : 512 * (s + 1)],
                                    start=(k == 0),
                                    stop=(k == 2),
                                )
                            osl = outsb[:, 512 * s : 512 * (s + 1)]
                            if t == 0:
                                nc.vector.tensor_scalar_add(
                                    out=osl, in0=pd, scalar1=biasv
                                )
                            else:
                                nc.vector.tensor_tensor(
                                    out=osl,
                                    in0=osl,
                                    in1=pd,
                                    op=mybir.AluOpType.add,
                                )

                nc.gpsimd.dma_start(
                    out=out_d[c * NB : (c + 1) * NB].rearrange(
                        "(one n) -> one n", one=1
                    ),
                    in_=outsb,
                )
    nc.finalize()
    return nc


def _build_weights(W1, b1, W2, b2, W3, b3, theta, bias):
    f16 = np.float16
    # A: 0/1 replication matrices. A[k, g, mcol] = 1 iff feature of row
    # (128g + mcol) is local feature k of f-tile g//12.
    gg, mm = np.meshgrid(np.arange(NG), np.arange(128), indexing="ij")
    r = 128 * gg + mm
    f = r // H1
    k = f - 128 * (gg // 12)
    A = np.zeros((128, NG, 128), dtype=f16)
    A[k.ravel(), gg.ravel(), mm.ravel()] = 1.0

    # W2blk[k, 2m+w, q]
    W2blk = np.zeros((128, 2 * NM, 128), dtype=f16)
    q = np.arange(128)
    f_s = q[None, :] * 0  # placeholder
    for m in range(NM):
        s = 128 * m + q
        f_col, j_col = s // H2, s % H2
        for w, g in enumerate(_g_pair(m)):
            rr = 128 * g + np.arange(128)
            f_row, i_row = rr // H1, rr % H1
            mask = f_row[:, None] == f_col[None, :]
            vals = W2[f_row[:, None], i_row[:, None], j_col[None, :]].astype(np.float32)
            W2blk[:, 2 * m + w, :] = np.where(mask, vals, 0.0).astype(f16)

    # W3blk[q, m, c]
    W3blk = np.zeros((128, NM, 32), dtype=f16)
    cc = np.arange(32)
    for m in range(NM):
        s = 128 * m + q
        f_col, j_col = s // H2, s % H2
        fo = 128 * (m // 8) + 32 * ((m % 8) // 2) + cc
        mask = f_col[:, None] == fo[None, :]
        vals = W3[f_col, j_col].astype(np.float32)[:, None]
        W3blk[:, m, :] = np.where(mask, vals, 0.0).astype(f16)

    w1v = np.ascontiguousarray(
        W1.astype(np.float32).ravel().reshape(NG, 128).T
    )
    b1v = np.ascontiguousarray(b1.astype(np.float32).ravel().reshape(NG, 128).T)
    b2v = np.ascontiguousarray(b2.astype(np.float32).ravel().reshape(NM, 128).T)
    b3v = np.ascontiguousarray(b3.astype(np.float32).reshape(NT, 128).T)
    thv = np.ascontiguousarray(theta.astype(np.float32).reshape(NT, 128).T)
    biasv = np.asarray(bias, dtype=np.float32).reshape(1, 1)
    return dict(
        Arep=A, W2blk=W2blk, W3blk=W3blk, w1v=w1v, b1v=b1v, b2v=b2v,
        b3v=b3v, thv=thv, biasv=biasv,
    )


def kernel(x, W1, b1, W2, b2, W3, b3, theta, bias):
    from concourse.bass_utils import run_bass_kernel_spmd

    x = np.asarray(x, dtype=np.float32)
    W1 = np.asarray(W1); b1 = np.asarray(b1); W2 = np.asarray(W2)
    b2 = np.asarray(b2); W3 = np.asarray(W3); b3 = np.asarray(b3)
    theta = np.asarray(theta); bias = np.asarray(bias)

    if "nc" not in _cache:
        _cache["nc"] = _build_nc()
    nc = _cache["nc"]

    wd = _build_weights(W1, b1, W2, b2, W3, b3, theta, bias)
    xT16 = np.ascontiguousarray(x.T.astype(np.float16))  # [F, B]

    in_maps = []
    for cidx in range(NCORES):
        m = dict(wd)
        m["xT16"] = np.ascontiguousarray(xT16[:, cidx * BS : (cidx + 1) * BS])
        in_maps.append(m)

    trace = bool(os.environ.get("BASS_KERNEL_TRACE"))
    res = run_bass_kernel_spmd(
        nc, in_maps, core_ids=list(range(NCORES)), trace=trace
    )
    _cache["last_exec_ns"] = res.exec_time_ns
    _cache["last_res"] = res

    Z = np.empty((B, F), dtype=np.float32)
    out = np.empty((B,), dtype=np.float32)
    for cidx in range(NCORES):
        r = res.results[cidx]
        Z[cidx * BS : (cidx + 1) * BS, :] = r["ZT"].T
        out[cidx * BS : (cidx + 1) * BS] = r["outv"]
    weights = np.ascontiguousarray(res.results[0]["wts"].T).reshape(F).astype(np.float32)
    return out, weights, Z
